# revision 4
# baseline (speedup 1.0000x reference)
"""Trainium2 Bass kernel for the GraphWalkAgent policy step.

Data-parallel over batch B=512 across 8 NeuronCores (64 rows each).
Embedding tables and MLP weights are replicated on every core.

Per core:
  - X = [E[cur] | hist | R[qr]]  (indirect row gathers + DMA)
  - X2 = relu(X@W1+b1)@W2+b2 on PE (inputs transposed via PE transposes,
    biases folded in as a K=1 ones-row matmul)
  - per-(b,a) action rows gathered from relation/entity tables with
    one-row-per-partition indirect DMAs (128 rows per instruction)
  - scores via fused DVE multiply + row-sum against a PE-broadcast X2 row
  - masked softmax (no max-subtraction needed: scores are O(1), masked
    entries are -1e31 and exp underflows to exactly 0) and entropy
    entropy = ln(Z) - sum(p * s)

Row layout on chip: global action row r = b*256 + a maps to SBUF
partition p = r // 128, slot j = r % 128, so b = p // 2 and all
per-b reductions are partition-pair combines done with small PE matmuls.
"""

import numpy as np

import concourse.bass as bass
import concourse.mybir as mybir
import concourse.tile as tile
from concourse import bacc
from concourse.bass import IndirectOffsetOnAxis
from concourse.bass_utils import run_bass_kernel_spmd
from concourse.masks import make_identity

N_CORES = 8
B, A = 512, 256
BL = B // N_CORES  # 64 rows per core
NUM_ENT, NUM_REL = 100000, 400
ENT_D, REL_D, HIST_D = 256, 256, 512
IN_D = ENT_D + HIST_D + REL_D  # 1024
ACT_D = ENT_D + REL_D  # 512
HUGE = 1e31
P = 128
NJ = (BL * A) // P  # 128 gather columns per core

F32 = mybir.dt.float32
I32 = mybir.dt.int32

_cached = {}


def build_module():
    nc = bacc.Bacc("TRN2", target_bir_lowering=False, debug=False,
                   num_devices=N_CORES)

    ent_emb = nc.dram_tensor("ent_emb", [NUM_ENT, ENT_D], F32, kind="ExternalInput")
    rel_emb = nc.dram_tensor("rel_emb", [NUM_REL, REL_D], F32, kind="ExternalInput")
    w1 = nc.dram_tensor("w1", [IN_D, ACT_D], F32, kind="ExternalInput")
    b1 = nc.dram_tensor("b1", [1, ACT_D], F32, kind="ExternalInput")
    w2 = nc.dram_tensor("w2", [ACT_D, ACT_D], F32, kind="ExternalInput")
    b2 = nc.dram_tensor("b2", [1, ACT_D], F32, kind="ExternalInput")
    ce_idx = nc.dram_tensor("ce_idx", [BL, 1], I32, kind="ExternalInput")
    qr_idx = nc.dram_tensor("qr_idx", [BL, 1], I32, kind="ExternalInput")
    hist = nc.dram_tensor("hist", [BL, HIST_D], F32, kind="ExternalInput")
    eidx = nc.dram_tensor("eidx", [P, NJ], I32, kind="ExternalInput")
    ridx = nc.dram_tensor("ridx", [P, NJ], I32, kind="ExternalInput")
    mask = nc.dram_tensor("mask", [P, NJ], I32, kind="ExternalInput")
    dist_out = nc.dram_tensor("dist_out", [P, NJ], F32, kind="ExternalOutput")
    ent_out = nc.dram_tensor("ent_out", [BL, 1], F32, kind="ExternalOutput")

    with tile.TileContext(nc) as tc:
        with (
            tc.tile_pool(name="const", bufs=1) as cpool,
            tc.tile_pool(name="wts", bufs=1) as wpool,
            tc.tile_pool(name="work", bufs=1) as spool,
            tc.tile_pool(name="grow", bufs=6) as gpool,
            tc.tile_pool(name="psum", bufs=2, space="PSUM") as ppool,
            tc.tile_pool(name="psmall", bufs=2, space="PSUM") as pspool,
            tc.tile_pool(name="pred", bufs=1, space="PSUM") as prpool,
        ):
            # ---- constants -------------------------------------------------
            ident = cpool.tile([P, P], F32)
            make_identity(nc, ident[:])
            ones = cpool.tile([1, BL], F32)
            nc.vector.memset(ones[:], 1.0)
            # sel[b, p] = 1 iff p//2 == b, built with two affine selects
            sel = cpool.tile([BL, P], F32)
            nc.gpsimd.memset(sel[:], 1.0)
            nc.gpsimd.affine_select(
                out=sel[:], in_=sel[:], compare_op=mybir.AluOpType.is_ge,
                fill=0.0, base=0, pattern=[[1, P]], channel_multiplier=-2)
            nc.gpsimd.affine_select(
                out=sel[:], in_=sel[:], compare_op=mybir.AluOpType.is_ge,
                fill=0.0, base=1, pattern=[[-1, P]], channel_multiplier=2)
            sel_ap = sel[:]

            # ---- small inputs ---------------------------------------------
            ce_t = spool.tile([BL, 1], I32)
            nc.sync.dma_start(out=ce_t[:], in_=ce_idx[:])
            qr_t = spool.tile([BL, 1], I32)
            nc.sync.dma_start(out=qr_t[:], in_=qr_idx[:])
            hist_t = spool.tile([BL, HIST_D], F32)
            nc.sync.dma_start(out=hist_t[:], in_=hist[:])
            eidx_t = spool.tile([P, NJ], I32)
            nc.sync.dma_start(out=eidx_t[:], in_=eidx[:])
            ridx_t = spool.tile([P, NJ], I32)
            nc.sync.dma_start(out=ridx_t[:], in_=ridx[:])
            mask_t = spool.tile([P, NJ], I32)
            nc.sync.dma_start(out=mask_t[:], in_=mask[:])
            b1_t = spool.tile([1, ACT_D], F32)
            nc.sync.dma_start(out=b1_t[:], in_=b1[:])
            b2_t = spool.tile([1, ACT_D], F32)
            nc.sync.dma_start(out=b2_t[:], in_=b2[:])

            # ---- weights: [K, M] blocks with K on partitions ---------------
            w1_t = wpool.tile([P, IN_D // P, ACT_D], F32)
            nc.sync.dma_start(
                out=w1_t[:], in_=w1[:].rearrange("(kc p) m -> p kc m", p=P))
            w2_t = wpool.tile([P, ACT_D // P, ACT_D], F32)
            nc.sync.dma_start(
                out=w2_t[:], in_=w2[:].rearrange("(kc p) m -> p kc m", p=P))

            # ---- gather the two MLP input embeddings (one row/partition) ---
            ecur_t = spool.tile([BL, ENT_D], F32)
            nc.gpsimd.indirect_dma_start(
                out=ecur_t[:], out_offset=None, in_=ent_emb[:],
                in_offset=IndirectOffsetOnAxis(ap=ce_t[:, 0:1], axis=0))
            rqr_t = spool.tile([BL, REL_D], F32)
            nc.gpsimd.indirect_dma_start(
                out=rqr_t[:], out_offset=None, in_=rel_emb[:],
                in_offset=IndirectOffsetOnAxis(ap=qr_t[:, 0:1], axis=0))

            # ---- X^T via PE transposes: [128, 8, 64] -----------------------
            xt = spool.tile([P, IN_D // P, BL], F32)
            srcs = ([(ecur_t, 0)] * 2 + [(hist_t, 2)] * 4 + [(rqr_t, 6)] * 2)
            for c in range(IN_D // P):
                src, base = srcs[c]
                pt = pspool.tile([P, BL], F32, space="PSUM", tag="tp")
                nc.tensor.transpose(
                    out=pt[:], in_=src[:BL, (c - base) * P:(c - base + 1) * P],
                    identity=ident[:BL, :BL])
                nc.vector.tensor_copy(out=xt[:, c, :], in_=pt[:])

            # ---- layer 1: H1 = relu(X @ W1 + b1)  ([64, 512] psum) ---------
            ph1 = ppool.tile([BL, ACT_D], F32, space="PSUM", tag="mlp")
            for kc in range(IN_D // P):
                nc.tensor.matmul(out=ph1[:], lhsT=xt[:, kc, :], rhs=w1_t[:, kc, :],
                                 start=(kc == 0), stop=False)
            nc.tensor.matmul(out=ph1[:], lhsT=ones[:1, :], rhs=b1_t[:1, :],
                             start=False, stop=True)
            h1 = spool.tile([BL, ACT_D], F32)
            nc.scalar.activation(out=h1[:], in_=ph1[:],
                                 func=mybir.ActivationFunctionType.Relu)

            # ---- H1^T ------------------------------------------------------
            h1t = spool.tile([P, ACT_D // P, BL], F32)
            for c in range(ACT_D // P):
                pt = pspool.tile([P, BL], F32, space="PSUM", tag="tp")
                nc.tensor.transpose(out=pt[:], in_=h1[:BL, c * P:(c + 1) * P],
                                    identity=ident[:BL, :BL])
                nc.vector.tensor_copy(out=h1t[:, c, :], in_=pt[:])

            # ---- layer 2: X2 = H1 @ W2 + b2 --------------------------------
            px2 = ppool.tile([BL, ACT_D], F32, space="PSUM", tag="mlp")
            for kc in range(ACT_D // P):
                nc.tensor.matmul(out=px2[:], lhsT=h1t[:, kc, :], rhs=w2_t[:, kc, :],
                                 start=(kc == 0), stop=False)
            nc.tensor.matmul(out=px2[:], lhsT=ones[:1, :], rhs=b2_t[:1, :],
                             start=False, stop=True)
            x2 = spool.tile([BL, ACT_D], F32)
            nc.scalar.copy(out=x2[:], in_=px2[:])

            # ---- broadcast X2 rows to partition pairs: bcast[p] = X2[p//2] -
            pb = ppool.tile([P, ACT_D], F32, space="PSUM", tag="bc")
            nc.tensor.matmul(out=pb[:], lhsT=sel_ap, rhs=x2[:BL, :],
                             start=True, stop=True)
            bcast = spool.tile([P, ACT_D], F32)
            nc.vector.tensor_copy(out=bcast[:], in_=pb[:])

            # sel2[p, b] = 1 iff p//2 == b  (sel transposed, materialized)
            pt = pspool.tile([P, BL], F32, space="PSUM", tag="tp")
            nc.tensor.transpose(out=pt[:], in_=sel_ap, identity=ident[:BL, :BL])
            sel2 = spool.tile([P, BL], F32)
            nc.vector.tensor_copy(out=sel2[:], in_=pt[:])

            # ---- main loop: gather [rel | ent] rows, fused dot -------------
            scores = spool.tile([P, NJ], F32)
            for j in range(NJ):
                grow = gpool.tile([P, ACT_D], F32, tag="grow")
                nc.gpsimd.indirect_dma_start(
                    out=grow[:, 0:REL_D], out_offset=None, in_=rel_emb[:],
                    in_offset=IndirectOffsetOnAxis(ap=ridx_t[:, j:j + 1], axis=0))
                nc.gpsimd.indirect_dma_start(
                    out=grow[:, REL_D:ACT_D], out_offset=None, in_=ent_emb[:],
                    in_offset=IndirectOffsetOnAxis(ap=eidx_t[:, j:j + 1], axis=0))
                nc.vector.scalar_tensor_tensor(
                    out=grow[:], in0=grow[:], scalar=1.0, in1=bcast[:],
                    op0=mybir.AluOpType.mult, op1=mybir.AluOpType.mult,
                    accum_out=scores[:, j:j + 1])

            # ---- mask: scores += (mask - 1) * HUGE -------------------------
            maskf = spool.tile([P, NJ], F32)
            nc.vector.tensor_copy(out=maskf[:], in_=mask_t[:])
            pen = spool.tile([P, NJ], F32)
            nc.vector.tensor_scalar(
                out=pen[:], in0=maskf[:], scalar1=-1.0, scalar2=HUGE,
                op0=mybir.AluOpType.add, op1=mybir.AluOpType.mult)
            nc.vector.tensor_tensor(out=scores[:], in0=scores[:], in1=pen[:],
                                    op=mybir.AluOpType.add)

            # ---- softmax + entropy -----------------------------------------
            pexp = spool.tile([P, NJ], F32)
            zcol = spool.tile([P, 1], F32)
            nc.scalar.activation(out=pexp[:], in_=scores[:],
                                 func=mybir.ActivationFunctionType.Exp,
                                 accum_out=zcol[:, 0:1])
            pz = prpool.tile([BL, 1], F32, space="PSUM", tag="red")
            nc.tensor.matmul(out=pz[:], lhsT=sel2[:], rhs=zcol[:],
                             start=True, stop=True)
            lnz = spool.tile([BL, 1], F32)
            nc.scalar.activation(out=lnz[:], in_=pz[:],
                                 func=mybir.ActivationFunctionType.Ln)
            rz = spool.tile([BL, 1], F32)
            nc.vector.reciprocal(out=rz[:], in_=pz[:])
            prb = prpool.tile([P, 1], F32, space="PSUM", tag="red2")
            nc.tensor.matmul(out=prb[:], lhsT=sel_ap, rhs=rz[:BL, :],
                             start=True, stop=True)
            rb = spool.tile([P, 1], F32)
            nc.vector.tensor_copy(out=rb[:], in_=prb[:])

            dist = spool.tile([P, NJ], F32)
            nc.vector.tensor_scalar(
                out=dist[:], in0=pexp[:], scalar1=rb[:, 0:1], scalar2=None,
                op0=mybir.AluOpType.mult)

            s1col = spool.tile([P, 1], F32)
            nc.vector.scalar_tensor_tensor(
                out=pen[:], in0=dist[:], scalar=1.0, in1=scores[:],
                op0=mybir.AluOpType.mult, op1=mybir.AluOpType.mult,
                accum_out=s1col[:, 0:1])
            ps1 = prpool.tile([BL, 1], F32, space="PSUM", tag="red")
            nc.tensor.matmul(out=ps1[:], lhsT=sel2[:], rhs=s1col[:],
                             start=True, stop=True)
            entv = spool.tile([BL, 1], F32)
            nc.vector.tensor_tensor(out=entv[:], in0=lnz[:], in1=ps1[:],
                                    op=mybir.AluOpType.subtract)

            nc.sync.dma_start(out=dist_out[:], in_=dist[:])
            nc.sync.dma_start(out=ent_out[:], in_=entv[:])

    nc.compile()
    return nc


def _get_module():
    if "nc" not in _cached:
        _cached["nc"] = build_module()
    return _cached["nc"]


def kernel(current_entity, query_relation, encoded_history, r_space, e_space,
           action_mask, entity_emb, relation_emb, W1, b1, W2, b2,
           _trace=False, _trace_kwargs=None):
    ce = np.asarray(current_entity).astype(np.int32)
    qr = np.asarray(query_relation).astype(np.int32)
    hist = np.asarray(encoded_history, dtype=np.float32)
    rsp = np.asarray(r_space).astype(np.int32)
    esp = np.asarray(e_space).astype(np.int32)
    am = np.asarray(action_mask).astype(np.int32)
    ent = np.ascontiguousarray(np.asarray(entity_emb, dtype=np.float32))
    rel = np.ascontiguousarray(np.asarray(relation_emb, dtype=np.float32))
    w1 = np.ascontiguousarray(np.asarray(W1, dtype=np.float32))
    w2 = np.ascontiguousarray(np.asarray(W2, dtype=np.float32))
    b1v = np.asarray(b1, dtype=np.float32).reshape(1, ACT_D)
    b2v = np.asarray(b2, dtype=np.float32).reshape(1, ACT_D)

    nc = _get_module()
    in_maps = []
    for c in range(N_CORES):
        s = slice(c * BL, (c + 1) * BL)
        in_maps.append({
            "ent_emb": ent,
            "rel_emb": rel,
            "w1": w1, "b1": b1v, "w2": w2, "b2": b2v,
            "ce_idx": np.ascontiguousarray(ce[s].reshape(BL, 1)),
            "qr_idx": np.ascontiguousarray(qr[s].reshape(BL, 1)),
            "hist": np.ascontiguousarray(hist[s]),
            "eidx": np.ascontiguousarray(esp[s].reshape(P, NJ)),
            "ridx": np.ascontiguousarray(rsp[s].reshape(P, NJ)),
            "mask": np.ascontiguousarray(am[s].reshape(P, NJ)),
        })

    kwargs = {}
    if _trace:
        kwargs["trace"] = True
        kwargs.update(_trace_kwargs or {})
    res = run_bass_kernel_spmd(nc, in_maps, core_ids=list(range(N_CORES)),
                               **kwargs)

    dist = np.concatenate(
        [r["dist_out"].reshape(BL, A) for r in res.results], axis=0)
    entropy = np.concatenate(
        [r["ent_out"].reshape(BL) for r in res.results], axis=0)
    if _trace:
        kernel._last_results = res
    return dist.astype(np.float32), entropy.astype(np.float32)
